# revision 1
# baseline (speedup 1.0000x reference)
"""Trainium2 Bass kernel for nn_BuildModel_3796751089795 (ON-LSTM-style RNN).

Model per reference:
  - sequential ON-LSTM cell over T=128 steps (cumax master gates L=3, CH=128)
  - per-step sliding-window (K=10) "theme"/"conv" head
  - output projection + per-batch-row selection at t = v_lengths[b]-1

Key algorithmic facts exploited:
  * only cur_output[b] = rnn[t_b, b] @ out_w + out_b is needed (t_b =
    v_lengths[b]-1), so the windowed conv/theme head is computed ONLY at t_b
    per batch row (gathered from the stored h/dist sequence), not at all T.
  * the x_t @ kernel_w part of the gate preactivation is computed inside the
    step loop by accumulating into the same PSUM tile as the h @ rec_w part
    (2 F-chunk matmuls against an on-chip transposed copy of X).
  * sigmoid(x) = 0.5*tanh(0.5x)+0.5 so every gate nonlinearity uses the one
    ACT table set that also contains exp (no per-step table switches).
  * local_dis softmax is computed on the gathered 10-tap dist window only.

Sharding: data-parallel over batch, B=512 -> 8 cores x 64 rows.

Self-contained: hardcodes all shapes; no file reads.
"""

import numpy as np
import ml_dtypes

import concourse.bass as bass
import concourse.tile as tile
from concourse import mybir
from concourse.bass_utils import run_bass_kernel_spmd
from concourse.masks import make_identity

F32 = mybir.dt.float32
F32R = mybir.dt.float32r
BF16 = mybir.dt.bfloat16
I32 = mybir.dt.int32
AF = mybir.ActivationFunctionType
OP = mybir.AluOpType
AX = mybir.AxisListType

B, T, F, H, L, K, LAB = 512, 128, 256, 384, 3, 10, 25
CH = H // L            # 128
GATES = 4 * H + 2 * L  # 1542
NCORES = 8
BL = B // NCORES       # 64 batch rows per core
PAD = K - 1            # 9 zero timesteps of h-prefix for window taps t<0

# pair-packed gate-column layout (see _prep_shared):
#   cols [0:512):     P-low  = [f_0 i_0 o_0 ci_0]      (chunk l=0)
#   cols [512:1024):  P-up   = [f_2 i_2 o_2 ci_2]      (chunk l=2)
#   cols [1024:1536): S      = [f_1 i_1 o_1 ci_1]      (chunk l=1)
#   cols [1536:1542): slot-low = [fm_pre(3) im_pre(3)]
#   cols [1542:1548): slot-up  = reversed slot-low (flip trick)
# The P tile is [128 part, 512]: lower partitions batch-rows for l=0 cols,
# upper partitions same batch-rows for l=2 cols. The upper slot columns are
# the reverse of the lower ones; running the identical softmax/cumsum chain
# on the upper half yields flipped quantities (a' = flip a, fm' = flip im,
# im' = flip fm), so the a column is correct for both halves directly and
# the b/d columns are written as four half-ops with the fm/im roles swapped
# on the upper half.
NG = 1536
NEWG = 1548


def _gate_perm_scale():
    """Return (perm, scale): new_W[:, j] = old_W[:, perm[j]] * scale[j]."""
    perm = np.zeros(NEWG, np.int64)
    scale = np.ones(NEWG, np.float32)

    def gcols(gi, l):
        return np.arange(2 * L + (gi * L + l) * CH, 2 * L + (gi * L + l + 1) * CH)

    FG, IG, OG, CIG = 0, 1, 2, 3
    blocks = [
        (0, [gcols(FG, 0), gcols(IG, 0), gcols(OG, 0), gcols(CIG, 0)]),
        (512, [gcols(FG, 2), gcols(IG, 2), gcols(OG, 2), gcols(CIG, 2)]),
        (1024, [gcols(FG, 1), gcols(IG, 1), gcols(OG, 1), gcols(CIG, 1)]),
    ]
    for base, cols in blocks:
        perm[base:base + 512] = np.concatenate(cols)
        scale[base:base + 3 * CH] = 0.5   # f/i/o tanh(x/2); ci stays
    perm[1536:1542] = np.arange(6)
    perm[1542:1548] = np.arange(6)[::-1]
    return perm, scale


def _prep_shared(kernel_w, kernel_b, rec_w, rec_b, scale_w, scale_b,
                 rescale_w, rescale_b, conv_w, conv_b, out_w, out_b):
    """Host-side weight preprocessing (replicated across cores).

    hseq stores hs = 2*h (h computed as (tanh(o/2)+1)*tanh(c) in one fused
    op); compensated by wrec *= 0.5 (recurrence), scale_w *= 0.5 (theme MLP
    input), conv_b *= 2 and out_w *= 0.5 (rnn' = 2*rnn)."""
    scale_w = scale_w * 0.5
    conv_b = conv_b * 2.0
    out_w = out_w * 0.5
    perm, colscale = _gate_perm_scale()

    def reorder(v):  # v [..., GATES]
        return (v[..., perm] * colscale).astype(np.float32)

    wpre = reorder(kernel_w[:F])                     # [256, 1548]
    wrec = reorder(rec_w[:H] * 0.5)                  # [384, 1548]; hs=2h comp.
    cb = reorder(kernel_b + rec_b + kernel_w[F] + rec_w[H])[None, :]   # [1,1548]
    trow = reorder(kernel_w[F] + rec_w[H])           # [1548]
    # trow2 [128, 1036]: cols 0:512 P-pair rows (low=l0 cols, up=l2 cols),
    # 512:1024 S rows (lower only), 1024:1030 slot-pair rows.
    trow2 = np.zeros((128, 1036), np.float32)
    trow2[0:64, 0:512] = trow[0:512]
    trow2[64:128, 0:512] = trow[512:1024]
    trow2[0:64, 512:1024] = trow[1024:1536]
    trow2[0:64, 1024:1030] = trow[1536:1542]
    trow2[64:128, 1024:1030] = trow[1542:1548]

    # conv_w [O=384, Hin=384, K=10] -> convT [128(h'), (k,hc,oc,o)] bf16
    convT = np.zeros((CH, K * 3 * 3 * CH), np.float32)
    for k in range(K):
        for hc in range(3):
            for oc in range(3):
                blk = conv_w[oc * CH:(oc + 1) * CH, hc * CH:(hc + 1) * CH, k].T
                off = ((k * 3 + hc) * 3 + oc) * CH
                convT[:, off:off + CH] = blk
    convT = convT.astype(ml_dtypes.bfloat16)

    scaleW = np.zeros((CH, 3 * 64), np.float32)      # lhsT chunks [h',feat]
    for hc in range(3):
        scaleW[:, hc * 64:(hc + 1) * 64] = scale_w[hc * CH:(hc + 1) * CH, :] / 10.0
    scalebT = scale_b.reshape(64, 1).astype(np.float32)
    rescaleW = rescale_w.astype(np.float32)          # [64, 384] = [K, (oc,o)]
    rescbT = np.zeros((CH, 3), np.float32)
    convbT = np.zeros((CH, 3), np.float32)
    outW = np.zeros((CH, 3 * LAB), np.float32)
    for oc in range(3):
        rescbT[:, oc] = 0.5 * rescale_b[oc * CH:(oc + 1) * CH]
        convbT[:, oc] = conv_b[oc * CH:(oc + 1) * CH]
        outW[:, oc * LAB:(oc + 1) * LAB] = out_w[oc * CH:(oc + 1) * CH, :]
    outb_rep = np.tile(out_b[None, :], (BL, 1)).astype(np.float32)

    # K=2 "indicator" lhsT: one bias matmul fills both partition halves of
    # the pair tile with their respective bias rows.
    ones2 = np.zeros((2, 128), np.float32)
    ones2[0, 0:64] = 1.0
    ones2[1, 64:128] = 1.0
    cbw2 = np.zeros((2, 518), np.float32)
    cbw2[0, 0:512] = cb[0, 0:512]
    cbw2[1, 0:512] = cb[0, 512:1024]
    cbw2[0, 512:518] = cb[0, 1536:1542]
    cbw2[1, 512:518] = cb[0, 1542:1548]

    return dict(
        wpre0=np.ascontiguousarray(wpre[:128]).astype(ml_dtypes.bfloat16),
        wpre1=np.ascontiguousarray(wpre[128:]).astype(ml_dtypes.bfloat16),
        wrec0=np.ascontiguousarray(wrec[:128]).astype(ml_dtypes.bfloat16),
        wrec1=np.ascontiguousarray(wrec[128:256]).astype(ml_dtypes.bfloat16),
        wrec2=np.ascontiguousarray(wrec[256:]).astype(ml_dtypes.bfloat16),
        cbw=cb.astype(np.float32), trow2=trow2, cbw2=cbw2,
        convT=convT, scaleW=scaleW, scalebT=scalebT,
        rescaleW=rescaleW, rescbT=rescbT, convbT=convbT,
        outW=outW, outb_rep=outb_rep,
        ones1=np.ones((1, BL), np.float32),
        ones2=ones2,
    )


def build_nc(t_steps=T, debug_hseq=False, reps=1):
    """Build the Bass module for one core (B-shard of 64 rows)."""
    nc = bass.Bass()
    ROWS = BL * t_steps
    HS_ROWS = (t_steps + PAD) * BL

    # ---------------- I/O ----------------
    d_x = nc.dram_tensor("x", [BL, t_steps, F], BF16, kind="ExternalInput")
    d_wpre = [nc.dram_tensor(f"wpre{i}", [128, NEWG], BF16, kind="ExternalInput")
              for i in range(2)]
    d_wrec = [nc.dram_tensor(f"wrec{i}", [128, NEWG], BF16, kind="ExternalInput")
              for i in range(3)]
    d_cbw = nc.dram_tensor("cbw", [1, NEWG], F32R, kind="ExternalInput")
    d_trow = nc.dram_tensor("trow2", [128, 1036], F32, kind="ExternalInput")
    d_convT = nc.dram_tensor("convT", [CH, K * 9 * CH], BF16, kind="ExternalInput")
    d_scaleW = nc.dram_tensor("scaleW", [CH, 192], F32, kind="ExternalInput")
    d_scaleb = nc.dram_tensor("scalebT", [64, 1], F32, kind="ExternalInput")
    d_rescaleW = nc.dram_tensor("rescaleW", [64, H], F32, kind="ExternalInput")
    d_rescb = nc.dram_tensor("rescbT", [CH, 3], F32, kind="ExternalInput")
    d_convb = nc.dram_tensor("convbT", [CH, 3], F32, kind="ExternalInput")
    d_outW = nc.dram_tensor("outW", [CH, 3 * LAB], F32, kind="ExternalInput")
    d_outb = nc.dram_tensor("outb_rep", [BL, LAB], F32, kind="ExternalInput")
    d_ones1 = nc.dram_tensor("ones1", [1, BL], F32R, kind="ExternalInput")
    d_ones2 = nc.dram_tensor("ones2", [2, 128], F32R, kind="ExternalInput")
    d_cbw2 = nc.dram_tensor("cbw2", [2, 518], F32R, kind="ExternalInput")
    d_gidx = nc.dram_tensor("gidx", [128, 5], I32, kind="ExternalInput")

    hseq_kind = "ExternalOutput" if debug_hseq else "Internal"
    d_hseq = nc.dram_tensor("hseq", [HS_ROWS, H + 1], F32, kind=hseq_kind)
    d_out = nc.dram_tensor("cur_out", [BL, LAB], F32, kind="ExternalOutput")
    d_dscr = nc.dram_tensor("dscr", [1, K * BL], F32)
    d_dbg_g = d_dbg_sm = None
    if debug_hseq:
        d_dbg_g = nc.dram_tensor("dbg_g", [t_steps * BL, NG], F32, kind="ExternalOutput")
        d_dbg_sm = nc.dram_tensor("dbg_sm", [t_steps * BL, 32], F32, kind="ExternalOutput")

    with tile.TileContext(nc) as tc:
        with (
            tc.tile_pool(name="singles", bufs=1) as singles,
            tc.tile_pool(name="post", bufs=1) as post_p,
        ):
            # ------------- load constants -------------
            ident = singles.tile([128, 128], F32)
            make_identity(nc, ident[:])
            id64 = ident[0:64, 0:64]
            identB = singles.tile([128, 128], BF16)
            make_identity(nc, identB[:])
            idB64 = identB[0:64, 0:64]
            wpre_sb = [singles.tile([128, NEWG], BF16, tag=f"wpre{i}", name=f"wpre{i}")
                       for i in range(2)]
            wrec_sb = [singles.tile([128, NEWG], BF16, tag=f"wrec{i}", name=f"wrec{i}")
                       for i in range(3)]
            for i in range(2):
                nc.sync.dma_start(wpre_sb[i][:], d_wpre[i][:])
            for i in range(3):
                nc.sync.dma_start(wrec_sb[i][:], d_wrec[i][:])
            cbw_sb = singles.tile([1, NEWG], F32R)
            nc.sync.dma_start(cbw_sb[:], d_cbw[:])
            trow_sb = singles.tile([128, 1036], F32)
            nc.sync.dma_start(trow_sb[:], d_trow[:])
            convT_sb = singles.tile([CH, K * 9 * CH], BF16)
            nc.sync.dma_start(convT_sb[:], d_convT[:])
            scaleW_sb = singles.tile([CH, 192], F32)
            nc.sync.dma_start(scaleW_sb[:], d_scaleW[:])
            scaleb_sb = singles.tile([64, 1], F32)
            nc.sync.dma_start(scaleb_sb[:], d_scaleb[:])
            rescaleW_sb = singles.tile([64, H], F32)
            nc.sync.dma_start(rescaleW_sb[:], d_rescaleW[:])
            rescb_sb = singles.tile([CH, 3], F32)
            nc.sync.dma_start(rescb_sb[:], d_rescb[:])
            convb_sb = singles.tile([CH, 3], F32)
            nc.sync.dma_start(convb_sb[:], d_convb[:])
            outW_sb = singles.tile([CH, 3 * LAB], F32)
            nc.sync.dma_start(outW_sb[:], d_outW[:])
            outb_sb = singles.tile([BL, LAB], F32)
            nc.sync.dma_start(outb_sb[:], d_outb[:])
            ones1_sb = singles.tile([1, BL], F32R)
            nc.sync.dma_start(ones1_sb[:], d_ones1[:])
            ones2_sb = singles.tile([2, 128], F32R)
            nc.sync.dma_start(ones2_sb[:], d_ones2[:])
            cbw2_sb = singles.tile([2, 518], F32R)
            nc.sync.dma_start(cbw2_sb[:], d_cbw2[:])
            sm_ring = singles.tile([128, 128], F32)
            nc.vector.memset(sm_ring[:], 1.0)   # cols 12,13 preset
            gidx_sb = singles.tile([128, 5], I32)
            nc.sync.dma_start(gidx_sb[:], d_gidx[:])

            # zero the h-prefix rows of hseq
            zrow = singles.tile([128, H + 1], F32)
            nc.vector.memset(zrow[:], 0.0)
            zr = PAD * BL  # 576
            for r0 in range(0, zr, 128):
                n = min(128, zr - r0)
                nc.sync.dma_start(d_hseq[r0:r0 + n, :], zrow[:n, :])

            # ------------- phase 1: X -> XT (F-major, bf16) -------------
            # XT[fc][f, t*64+b] = X[b, t, f]. Production is interleaved into
            # the recurrence loop (one rt chunk per step, 2x the consumption
            # rate) so the transposition rides in engine slack instead of
            # serializing ~100us up front.
            xt_sb = [singles.tile([128, ROWS], BF16, tag=f"xt{i}", name=f"xt{i}")
                     for i in range(2)]
            x_tmaj = d_x[:].rearrange("b t f -> t b f")
            NRT = ROWS // 128
            XPREF = 6

            # ------------- phase 2: recurrence (pair-packed) -------------
            # P psum tile [128,512] holds chunk l=0 gates in the lower 64
            # partitions and chunk l=2 gates (f/i swapped) in the upper 64;
            # S [64,512] holds chunk l=1. One tanh covers l0+l2. The slot /
            # sm chain runs on [128,*] tiles with column-reversed weights on
            # the upper half, which makes every scalar column [128,1] carry
            # (a_0|a_2) etc. h is stored as hs = 2h = (to+1)*tanh(c); hT is
            # produced H-major: hT = (toT+1)*tcT from bf16 transposes.
            # hT column blocks: hc0 -> 0:64, hc2 -> 64:128, hc1 -> 128:192.
            PLO, PUP, SC = slice(0, 512), slice(512, 1024), slice(1024, 1536)
            SLO, SUP = slice(1536, 1542), slice(1542, 1548)
            HTC = {0: 0, 2: 64, 1: 128}   # hT col offset per h chunk
            for rep_i in range(reps):
                with (
                    tc.tile_pool(name="xo", bufs=2, space="PSUM") as xo_p,
                    tc.tile_pool(name="smallps", bufs=1, space="PSUM") as smallps_p,
                    tc.tile_pool(name="gates", bufs=2) as gates_p,
                    tc.tile_pool(name="state", bufs=2) as state_p,
                    tc.tile_pool(name="wk", bufs=2) as wk_p,
                    tc.tile_pool(name="xrow", bufs=4) as xrow_p,
                ):
                    fmim_ring = smallps_p.tile([128, 512], F32, tag="fmim", name="fmim")
                    tr_ring = smallps_p.tile([128, 768], BF16, tag="trring",
                                             name="trring")
                    xtp_ring = smallps_p.tile([128, 512], BF16, tag="xtp",
                                              name="xtp_ring")

                    def do_rt(rt):
                        """Transpose one [128-row] chunk of X into xt_sb."""
                        xr = xrow_p.tile([128, F], BF16, tag="xrow", name=f"xr{rt}")
                        t0 = rt * 2
                        nc.sync.dma_start(xr[0:64, :], x_tmaj[t0, :, :])
                        nc.sync.dma_start(xr[64:128, :], x_tmaj[t0 + 1, :, :])
                        xo_ = (rt % 2) * 256
                        for fc in range(2):
                            pt = xtp_ring[:, xo_ + fc * 128:xo_ + (fc + 1) * 128]
                            nc.tensor.transpose(pt, xr[:, fc * 128:(fc + 1) * 128],
                                                identB[:])
                            nc.scalar.copy(
                                xt_sb[fc][:, rt * 128:(rt + 1) * 128], pt)

                    for rt in range(min(XPREF, NRT)):
                        do_rt(rt)

                    def slot_ap(t, lo, hi):
                        o = (t % 64) * 8
                        return fmim_ring[lo:hi, o:o + 6]

                    def x_mms(t, P, S, part, stop):
                        """Issue bias + x matmuls for one accumulation group.
                        Pair bias fills both halves in ONE K=2 indicator mm."""
                        if part == "slots":
                            nc.tensor.matmul(slot_ap(t, 0, 128), ones2_sb[:],
                                             cbw2_sb[:, 512:518],
                                             start=True, stop=False)
                            for (lo, hi), cs in (((0, 64), SLO), ((64, 128), SUP)):
                                sl = slot_ap(t, lo, hi)
                                for fc in range(2):
                                    nc.tensor.matmul(
                                        sl, xt_sb[fc][:, t * BL:(t + 1) * BL],
                                        wpre_sb[fc][:, cs], start=False,
                                        stop=(stop and fc == 1))
                            return
                        if part == "pbias":
                            nc.tensor.matmul(P[:], ones2_sb[:], cbw2_sb[:, 0:512],
                                             start=True, stop=False)
                            return
                        if part == "s":
                            nc.tensor.matmul(S[:], ones1_sb[:], cbw_sb[:, SC],
                                             start=True, stop=False)
                            for fc in range(2):
                                nc.tensor.matmul(
                                    S[:], xt_sb[fc][:, t * BL:(t + 1) * BL],
                                    wpre_sb[fc][:, SC], start=False,
                                    stop=(stop and fc == 1))
                            return
                        out, cs = {"plo": (P[0:64, :], PLO),
                                   "pup": (P[64:128, :], PUP)}[part]
                        for fc in range(2):
                            nc.tensor.matmul(
                                out, xt_sb[fc][:, t * BL:(t + 1) * BL],
                                wpre_sb[fc][:, cs], start=False,
                                stop=(stop and fc == 1))

                    def new_ps(t):
                        P = xo_p.tile([128, 512], F32, tag="P", name=f"P_{t}")
                        S = xo_p.tile([BL, 512], F32, tag="S", name=f"S_{t}")
                        return P, S

                    P_cur, S_cur = new_ps(0)
                    x_mms(0, P_cur, S_cur, "slots", stop=True)
                    x_mms(0, P_cur, S_cur, "pbias", stop=False)
                    x_mms(0, P_cur, S_cur, "plo", stop=False)
                    x_mms(0, P_cur, S_cur, "pup", stop=True)
                    x_mms(0, P_cur, S_cur, "s", stop=True)

                    hT_prev = None    # [128, 192] bf16 SBUF
                    cP_prev = None    # [128, 128] bf16 (lower c_0, upper c_2)
                    cS_prev = None    # [64, 128] bf16 (c_1)

                    for t in range(t_steps):
                        # ---- h matmuls. All slot h-mms FIRST with their stop, so
                        # the slot group completes ~60ns in and exp/sm run
                        # concurrently with the big h-mms. hc-blocks in hT
                        # production order (0+2 from the pair stt, then 1). ----
                        if t > 0:
                            for hc in (0, 2, 1):
                                hTs = hT_prev[:, HTC[hc]:HTC[hc] + 64]
                                for (lo, hi), cs in (((0, 64), SLO), ((64, 128), SUP)):
                                    nc.tensor.matmul(slot_ap(t, lo, hi), hTs,
                                                     wrec_sb[hc][:, cs],
                                                     start=False, stop=(hc == 1))
                            for hc in (0, 2, 1):
                                last = hc == 1
                                hTs = hT_prev[:, HTC[hc]:HTC[hc] + 64]
                                nc.tensor.matmul(P_cur[0:64, :], hTs,
                                                 wrec_sb[hc][:, PLO],
                                                 start=False, stop=False)
                                nc.tensor.matmul(P_cur[64:128, :], hTs,
                                                 wrec_sb[hc][:, PUP],
                                                 start=False, stop=last)
                                nc.tensor.matmul(S_cur[:], hTs,
                                                 wrec_sb[hc][:, SC],
                                                 start=False, stop=last)
                        else:
                            # t=0: Tint=0 -> remove trow part of the bias row
                            nc.vector.tensor_tensor(P_cur[:], P_cur[:],
                                                    trow_sb[:, 0:512], op=OP.subtract)
                            nc.vector.tensor_tensor(S_cur[:], S_cur[:],
                                                    trow_sb[0:64, 512:1024],
                                                    op=OP.subtract)
                            nc.vector.tensor_tensor(
                                fmim_ring[:, 0:6], fmim_ring[:, 0:6],
                                trow_sb[:, 1024:1030], op=OP.subtract)

                        # ---- sm chain: exp (Act) + DVE small ops on [128,*] ----
                        # cols: 0:6 e (1->u01, 4->u45), 6:8 sums, 8:10 r,
                        # 10:13 fm trio, 13:16 im trio (12,13 preset 1),
                        # 16:19 s1, 19:22 a, 22:25 b, 25:28 d, 28 fm0+fm1
                        so = (t % 4) * 32
                        sm = sm_ring[:, so:so + 32]
                        o = (t % 64) * 8
                        nc.scalar.activation(sm[:, 0:6], fmim_ring[:, o:o + 6], AF.Exp)
                        nc.vector.tensor_reduce(
                            sm[:, 6:8], sm[:, 0:6].rearrange("p (a b) -> p a b", b=3),
                            axis=AX.X, op=OP.add)
                        nc.vector.tensor_tensor(sm[:, 1:2], sm[:, 1:2], sm[:, 0:1],
                                                op=OP.add)   # u01
                        nc.vector.tensor_tensor(sm[:, 4:5], sm[:, 4:5], sm[:, 5:6],
                                                op=OP.add)   # u45
                        nc.vector.reciprocal(sm[:, 8:10], sm[:, 6:8])
                        nc.vector.tensor_scalar(sm[:, 10:12], sm[:, 0:2],
                                                scalar1=sm[:, 8:9], scalar2=None,
                                                op0=OP.mult)   # fm0, fm1
                        nc.vector.tensor_scalar(sm[:, 14:16], sm[:, 4:6],
                                                scalar1=sm[:, 9:10], scalar2=None,
                                                op0=OP.mult)   # im1, im2
                        nc.vector.tensor_tensor(sm[:, 16:19], sm[:, 10:13],
                                                sm[:, 13:16], op=OP.mult)   # s1
                        nc.vector.tensor_scalar(sm[:, 19:22], sm[:, 16:19],
                                                scalar1=0.5, scalar2=None,
                                                op0=OP.mult)   # a
                        # b cols hold [b_l | b_{2-l}], d cols [d_l | d_{2-l}]:
                        # lower halves use the (fm, im) formulas, upper halves
                        # the swapped ones (upper chain is the flipped softmax).
                        nc.vector.scalar_tensor_tensor(
                            sm[0:64, 22:25], sm[0:64, 16:19], -0.5, sm[0:64, 10:13],
                            op0=OP.mult, op1=OP.add)            # b low = fm - a
                        nc.vector.scalar_tensor_tensor(
                            sm[64:128, 22:25], sm[64:128, 16:19], -0.5,
                            sm[64:128, 13:16], op0=OP.mult, op1=OP.add)
                        nc.vector.scalar_tensor_tensor(
                            sm[0:64, 25:28], sm[0:64, 16:19], -0.5, sm[0:64, 13:16],
                            op0=OP.mult, op1=OP.add)            # d low = im - a
                        nc.vector.scalar_tensor_tensor(
                            sm[64:128, 25:28], sm[64:128, 16:19], -0.5,
                            sm[64:128, 10:13], op0=OP.mult, op1=OP.add)

                        P_nx, S_nx = (new_ps(t + 1) if t + 1 < t_steps
                                      else (None, None))

                        gP = gates_p.tile([128, 512], BF16, tag="gP", name=f"gP_{t}")
                        gS = gates_p.tile([BL, 512], BF16, tag="gS", name=f"gS_{t}")
                        cP = state_p.tile([128, 128], BF16, tag="cP", name=f"cP_{t}")
                        cS = state_p.tile([BL, 128], BF16, tag="cS", name=f"cS_{t}")
                        tcP = wk_p.tile([128, 128], BF16, tag="tcP", name=f"tcP_{t}")
                        tcS = wk_p.tile([BL, 128], BF16, tag="tcS", name=f"tcS_{t}")
                        hout = state_p.tile([BL, H + 1], F32, tag="hout",
                                            name=f"hout_{t}")
                        hT_new = (state_p.tile([128, 192], BF16, tag="hT",
                                               name=f"hT_{t}")
                                  if t + 1 < t_steps else None)
                        tro = (t % 2) * 384

                        a_c = sm[:, 19:20]
                        b_c = sm[:, 22:23]
                        d_c = sm[:, 25:26]
                        a1 = sm[0:64, 20:21]
                        b1 = sm[0:64, 23:24]
                        d1 = sm[0:64, 26:27]

                            # Priority order: the pair-chunk tail gates the next
                        # step's first h-mms (hc0/hc2), so it runs first on every
                        # engine; the S (l=1) tail has ~0.9us of slack before the
                        # hc1 block needs it.
                        hsP = wk_p.tile([128, 128], BF16, tag="hsP", name=f"hsP_{t}")
                        hsS = wk_p.tile([BL, 128], BF16, tag="hsS", name=f"hsS_{t}")
                        FvC = wk_p.tile([128, 128], BF16, tag="FvC", name=f"FvC_{t}")
                        IvC = wk_p.tile([128, 128], BF16, tag="IvC", name=f"IvC_{t}")
                        uvP = wk_p.tile([128, 128], BF16, tag="uvP", name=f"uvP_{t}")
                        nc.scalar.activation(gP[:], P_cur[:], AF.Tanh)
                        nc.vector.tensor_scalar(IvC[:], gP[:, 128:256], scalar1=a_c,
                                                scalar2=d_c, op0=OP.mult, op1=OP.add)
                        if t > 0:
                            nc.vector.tensor_scalar(FvC[:], gP[:, 0:128],
                                                    scalar1=a_c, scalar2=b_c,
                                                    op0=OP.mult, op1=OP.add)
                            nc.vector.tensor_tensor(uvP[:], FvC[:], cP_prev[:],
                                                    op=OP.mult)
                            nc.vector.tensor_tensor(cP[:], IvC[:], gP[:, 384:512],
                                                    op=OP.mult)
                            nc.vector.tensor_tensor(cP[:], cP[:], uvP[:], op=OP.add)
                        else:
                            nc.vector.tensor_tensor(cP[:], IvC[:], gP[:, 384:512],
                                                    op=OP.mult)
                        with tc.high_priority():
                            nc.scalar.activation(tcP[:], cP[:], AF.Tanh)
                            nc.vector.scalar_tensor_tensor(
                                hsP[:], gP[:, 256:384], 1.0, tcP[:],
                                op0=OP.add, op1=OP.mult)
                        nc.scalar.activation(gS[:], S_cur[:], AF.Tanh)
                        if P_nx is not None:
                            x_mms(t + 1, P_nx, S_nx, "slots", stop=False)
                            x_mms(t + 1, P_nx, S_nx, "pbias", stop=False)
                            x_mms(t + 1, P_nx, S_nx, "plo", stop=False)
                            x_mms(t + 1, P_nx, S_nx, "pup", stop=False)
                            x_mms(t + 1, P_nx, S_nx, "s", stop=False)
                            with tc.high_priority():
                                nc.tensor.transpose(
                                    tr_ring[:, tro:tro + 128], hsP[:],
                                    identB[:])
                                nc.vector.tensor_copy(
                                    hT_new[:, 0:128],
                                    tr_ring[:, tro:tro + 128])
                        Fv1 = wk_p.tile([BL, 128], BF16, tag="Fv1", name=f"Fv1_{t}")
                        Iv1 = wk_p.tile([BL, 128], BF16, tag="Iv1", name=f"Iv1_{t}")
                        uv1 = wk_p.tile([BL, 128], BF16, tag="uv1", name=f"uv1_{t}")
                        nc.vector.tensor_scalar(Iv1[:], gS[:, 128:256], scalar1=a1,
                                                scalar2=d1, op0=OP.mult, op1=OP.add)
                        if t > 0:
                            nc.vector.tensor_scalar(Fv1[:], gS[:, 0:128], scalar1=a1,
                                                    scalar2=b1, op0=OP.mult,
                                                    op1=OP.add)
                            nc.vector.tensor_tensor(uv1[:], Fv1[:], cS_prev[:],
                                                    op=OP.mult)
                            nc.vector.tensor_tensor(cS[:], Iv1[:], gS[:, 384:512],
                                                    op=OP.mult)
                            nc.vector.tensor_tensor(cS[:], cS[:], uv1[:], op=OP.add)
                        else:
                            nc.vector.tensor_tensor(cS[:], Iv1[:], gS[:, 384:512],
                                                    op=OP.mult)
                        nc.scalar.activation(tcS[:], cS[:], AF.Tanh)
                        nc.vector.scalar_tensor_tensor(
                            hsS[:], gS[:, 256:384], 1.0, tcS[:],
                            op0=OP.add, op1=OP.mult)
                        if P_nx is not None:
                            nc.tensor.transpose(tr_ring[:, tro + 128:tro + 192],
                                                hsS[:], idB64)
                            nc.vector.tensor_copy(hT_new[:, 128:192],
                                                  tr_ring[:, tro + 128:tro + 192])

                        # B-major hout rows for the hseq DMA + dist. The
                        # upper->lower partition shift must run on DVE (gpsimd
                        # Q7 cores are partition-local).
                        nc.gpsimd.tensor_copy(hout[:, 0:128], hsP[0:64, :])
                        nc.gpsimd.tensor_copy(hout[:, 128:256], hsS[:])
                        nc.vector.tensor_copy(hout[:, 256:384], hsP[64:128, :])
                        nc.gpsimd.tensor_tensor(sm[0:64, 28:29], sm[0:64, 10:11],
                                                sm[0:64, 11:12], op=OP.add)
                        nc.gpsimd.tensor_scalar(
                            hout[:, H:H + 1], sm[0:64, 28:29],
                            scalar1=-1.0 / 3.0, scalar2=2.0 / 3.0,
                            op0=OP.mult, op1=OP.add)

                        nc.sync.dma_start(d_hseq[(t + PAD) * BL:(t + PAD + 1) * BL, :],
                                          hout[:])
                        if debug_hseq:
                            nc.sync.dma_start(d_dbg_sm[t * BL:(t + 1) * BL, :],
                                              sm_ring[0:64, 0:32])
                        if t + XPREF < NRT:
                            do_rt(t + XPREF)

                        hT_prev = hT_new
                        P_cur, S_cur = P_nx, S_nx
                        cP_prev, cS_prev = cP, cS

                # ------------- phase 3: windowed head at t_b only -------------
                with (
                    tc.tile_pool(name="postps", bufs=1, space="PSUM") as postps_p,
                    tc.tile_pool(name="postps2", bufs=2, space="PSUM") as postps2_p,
                ):
                    gath = [post_p.tile([128, H + 1], F32, tag=f"gath{j}", name=f"gath{j}")
                            for j in range(5)]
                    for j in range(5):
                        nc.gpsimd.indirect_dma_start(
                            out=gath[j][:], out_offset=None, in_=d_hseq[:],
                            in_offset=bass.IndirectOffsetOnAxis(ap=gidx_sb[:, j:j + 1],
                                                                axis=0))

                    # dist window [64, 10] -> cumsum -> softmax -> d_win
                    ww = post_p.tile([BL, 16], F32, name="ww")
                    cum = post_p.tile([BL, 64], F32, name="cum")
                    for k in range(K):
                        j, par = k // 2, k % 2
                        nc.vector.tensor_copy(ww[:, k:k + 1],
                                              gath[j][par * 64:par * 64 + 64, H:H + 1])
                    nc.vector.tensor_copy(cum[:, 0:1], ww[:, 0:1])
                    nc.vector.tensor_tensor(cum[:, 1:10], ww[:, 1:10], ww[:, 0:9], op=OP.add)
                    nc.vector.tensor_copy(cum[:, 16:18], cum[:, 0:2])
                    nc.vector.tensor_tensor(cum[:, 18:26], cum[:, 2:10], cum[:, 0:8],
                                            op=OP.add)
                    nc.vector.tensor_copy(cum[:, 32:36], cum[:, 16:20])
                    nc.vector.tensor_tensor(cum[:, 36:42], cum[:, 20:26], cum[:, 16:22],
                                            op=OP.add)
                    nc.vector.tensor_copy(cum[:, 48:56], cum[:, 32:40])
                    nc.vector.tensor_tensor(cum[:, 56:58], cum[:, 40:42], cum[:, 32:34],
                                            op=OP.add)
                    nmx = post_p.tile([BL, 4], F32, name="nmx")
                    nc.vector.tensor_reduce(nmx[:, 0:1], cum[:, 48:58], axis=AX.X,
                                            op=OP.max, negate=True)
                    dwin = post_p.tile([BL, 16], F32, name="dwin")
                    nc.scalar.activation(dwin[:, 0:10], cum[:, 48:58], AF.Exp,
                                         bias=nmx[:, 0:1], scale=1.0)
                    nc.vector.tensor_reduce(nmx[:, 1:2], dwin[:, 0:10], axis=AX.X,
                                            op=OP.add)
                    nc.vector.reciprocal(nmx[:, 2:3], nmx[:, 1:2])
                    nc.vector.tensor_scalar(dwin[:, 0:10], dwin[:, 0:10],
                                            scalar1=nmx[:, 2:3], scalar2=None, op0=OP.mult)

                    # d_win -> [1, 640] k-major -> broadcast [128, 640]
                    dT_ps = postps_p.tile([K, BL], F32, tag="dTps", name="dT_ps")
                    nc.tensor.transpose(dT_ps[:], dwin[:, 0:K], id64)
                    dT = post_p.tile([K, BL], F32, name="dT")
                    nc.vector.tensor_copy(dT[:], dT_ps[:])
                    nc.sync.dma_start(d_dscr[:], dT[:])
                    dbc = post_p.tile([128, K * BL], F32, name="dbc")
                    nc.gpsimd.dma_start(dbc[:], d_dscr[:].to_broadcast([128, K * BL]))

                    # gathered h -> transposed chunks gathT[hc][h', k*64+b]
                    gathT = [post_p.tile([128, K * BL], F32, tag=f"gathT{hc}",
                                         name=f"gathT{hc}") for hc in range(3)]
                    for j in range(5):
                        for hc in range(3):
                            pt = postps2_p.tile([128, 128], F32, tag="postTp", name="pt2")
                            nc.tensor.transpose(pt[:], gath[j][:, hc * 128:(hc + 1) * 128],
                                                ident[:])
                            nc.scalar.copy(gathT[hc][:, j * 128:(j + 1) * 128], pt[:])

                    # weighted taps (bf16) for conv + theme
                    wg = [post_p.tile([128, K * BL], BF16, tag=f"wg{hc}", name=f"wg{hc}")
                          for hc in range(3)]
                    for hc in range(3):
                        nc.vector.tensor_tensor(wg[hc][:], gathT[hc][:], dbc[:], op=OP.mult)

                    # theme_in.T chunks = sum_k wg (scale_w already /10)
                    thin = [post_p.tile([128, BL], F32, tag=f"thin{hc}", name=f"thin{hc}")
                            for hc in range(3)]
                    for hc in range(3):
                        nc.vector.tensor_reduce(
                            thin[hc][:], wg[hc][:].rearrange("p (k b) -> p b k", b=BL),
                            axis=AX.X, op=OP.add)

                    # MLP: u = thin @ scaleW + b -> relu -> v = ru @ rescaleW -> sigmoid
                    u_ps = postps_p.tile([64, BL], F32, tag="ups", name="u_ps")
                    for hc in range(3):
                        nc.tensor.matmul(u_ps[:], scaleW_sb[:, hc * 64:(hc + 1) * 64],
                                         thin[hc][:], start=(hc == 0), stop=(hc == 2))
                    ru = post_p.tile([64, BL], F32, name="ru")
                    nc.vector.tensor_scalar(ru[:], u_ps[:], scalar1=scaleb_sb[:, 0:1],
                                            scalar2=0.0, op0=OP.add, op1=OP.max)
                    th = [post_p.tile([128, BL], F32, tag=f"th{oc}", name=f"th{oc}")
                          for oc in range(3)]
                    for oc in range(3):
                        v_ps = postps_p.tile([128, BL], F32, tag="vps", name="v_ps")
                        nc.tensor.matmul(v_ps[:], rescaleW_sb[:, oc * 128:(oc + 1) * 128],
                                         ru[:], start=True, stop=True)
                        nc.scalar.activation(th[oc][:], v_ps[:], AF.Tanh,
                                             bias=rescb_sb[:, oc:oc + 1], scale=0.5)
                        nc.vector.tensor_scalar(th[oc][:], th[oc][:], scalar1=0.5,
                                                scalar2=0.5, op0=OP.mult, op1=OP.add)

                    # conv.T[oc] = sum_{k,hc} convT_k_hc_oc.T @ wg[hc][:, k-slice]
                    rnnT = [post_p.tile([128, BL], F32, tag=f"rnnT{oc}", name=f"rnnT{oc}")
                            for oc in range(3)]
                    for oc in range(3):
                        cv_ps = postps2_p.tile([128, BL], F32, tag="cvps", name="cv_ps")
                        n = 0
                        for k in range(K):
                            for hc in range(3):
                                off = ((k * 3 + hc) * 3 + oc) * CH
                                nc.tensor.matmul(
                                    cv_ps[:], convT_sb[:, off:off + CH],
                                    wg[hc][:, k * BL:(k + 1) * BL],
                                    start=(n == 0), stop=(n == 3 * K - 1))
                                n += 1
                        # (conv + conv_b) * theme + h_sel
                        nc.vector.tensor_scalar(rnnT[oc][:], cv_ps[:],
                                                scalar1=convb_sb[:, oc:oc + 1],
                                                scalar2=None, op0=OP.add)
                        nc.vector.tensor_tensor(rnnT[oc][:], rnnT[oc][:], th[oc][:],
                                                op=OP.mult)
                        nc.vector.tensor_tensor(rnnT[oc][:], rnnT[oc][:],
                                                gathT[oc][:, (K - 1) * BL:K * BL],
                                                op=OP.add)

                    # output projection + bias
                    o_ps = postps_p.tile([BL, LAB], F32, tag="ops", name="o_ps")
                    for hc in range(3):
                        nc.tensor.matmul(o_ps[:], rnnT[hc][:],
                                         outW_sb[:, hc * LAB:(hc + 1) * LAB],
                                         start=(hc == 0), stop=(hc == 2))
                    ofin = post_p.tile([BL, LAB], F32, name="ofin")
                    nc.vector.tensor_tensor(ofin[:], o_ps[:], outb_sb[:], op=OP.add)
                    nc.sync.dma_start(d_out[:], ofin[:])

    _split_drain_waits(nc)
    return nc


def _split_drain_waits(nc, limit=1):
    """Workaround: this walrus rejects instructions carrying more than
    `limit` semaphore waits; hoist excess waits onto NoOps just before."""
    n = 0
    for fn in nc.m.functions:
        for bb in fn.blocks:
            new_insts = []
            for inst in bb.instructions:
                si = inst.sync_info
                if si and si.on_wait and len(si.on_wait) > limit:
                    waits = list(si.on_wait)
                    for w in waits[limit:]:
                        n += 1
                        nop = mybir.InstNoOp(name=f"I-dsplit-{n}", ins=[], outs=[])
                        nop.engine = inst.engine
                        nop.sync_info = mybir.SyncInfo(on_wait=[w], on_update=[])
                        new_insts.append(nop)
                    inst.sync_info = mybir.SyncInfo(on_wait=waits[:limit],
                                                    on_update=list(si.on_update))
                new_insts.append(inst)
            bb.instructions = new_insts
    return n


def _make_inmaps(inputs, t_steps=T, ncores=NCORES):
    X = np.asarray(inputs["X"], np.float32)
    v_lengths = np.asarray(inputs["v_lengths"]).astype(np.int64)
    shared = _prep_shared(
        np.asarray(inputs["kernel_w"], np.float32), np.asarray(inputs["kernel_b"], np.float32),
        np.asarray(inputs["rec_w"], np.float32), np.asarray(inputs["rec_b"], np.float32),
        np.asarray(inputs["scale_w"], np.float32), np.asarray(inputs["scale_b"], np.float32),
        np.asarray(inputs["rescale_w"], np.float32), np.asarray(inputs["rescale_b"], np.float32),
        np.asarray(inputs["conv_w"], np.float32), np.asarray(inputs["conv_b"], np.float32),
        np.asarray(inputs["out_w"], np.float32), np.asarray(inputs["out_b"], np.float32))
    in_maps = []
    for c in range(ncores):
        bs = slice(c * BL, (c + 1) * BL)
        vl = v_lengths[bs]
        gidx = np.zeros((128, 5), np.int32)
        for p in range(128):
            for j in range(5):
                b = p % 64
                k = 2 * j + p // 64
                tb = int(vl[b]) - 1
                gidx[p, j] = (tb + k) * BL + b
        m = dict(shared)
        m["x"] = np.ascontiguousarray(X[bs, :t_steps, :]).astype(ml_dtypes.bfloat16)
        m["gidx"] = gidx
        in_maps.append(m)
    return in_maps


_NC_CACHE = {}


def kernel(**inputs) -> np.ndarray:
    t_steps = T
    if t_steps not in _NC_CACHE:
        _NC_CACHE[t_steps] = build_nc(t_steps)
    nc = _NC_CACHE[t_steps]
    in_maps = _make_inmaps(inputs, t_steps)
    res = run_bass_kernel_spmd(nc, in_maps, list(range(NCORES)))
    out = np.concatenate([res.results[c]["cur_out"] for c in range(NCORES)], axis=0)
    return out.astype(np.float32)

